# revision 19
# baseline (speedup 1.0000x reference)
"""Self-attention kernel for Trainium2 (8 NeuronCores, SPMD).

Problem: X[8192,512], Wq,Wk[512,512]:
    Q = X@Wq ; K = X@Wk ; S = softmax(Q K^T / sqrt(512)) ; out = S @ X

Sharding: rows of Q (query blocks of 1024) across 8 cores; K/V (=X) replicated.

Per-core dataflow (core owns query rows i in [c*1024, (c+1)*1024)):
  Phase P:  M^T = (Wq Wk^T)/sqrt(d)  [512,512]  (16 MMs, f32r, pipelined
            against the 4-chunk weights DMA via 4 concurrent PSUM groups)
            R   = M X_mine^T          [512,1024] (32 MMs; S^T = X R)
  Per i-half h (512 query columns):
    B1: for each j-tile (64): S^T tile [j=128, i=512] = sum_e X^T_tile.T R
        (4 accumulating f32r matmuls) -> ACT copy PSUM->SBUF,
        DVE running elementwise max -> mx[128,512]
    B2: partition-reduce mx via PE transpose + DVE reduce_max -> m[1,512];
        broadcast back to [128,512] via ones outer-product matmul
    B3: for each j-tile: d = S^T_t - B (DVE), exp (ACT, f16 out) -> P~;
        PE per i-chunk c: o[c,512] += P~[:,c].T @ X[j-tile] (N=512 fp16 MM)
        + a free N=1 MM on the same stationary:
        sum[c-chunk i,1] += P~[:,c].T @ ones  (row sums, already transposed)
    B4: DVE recip(sum[128,4]) -> per-bank scaled drain copies split across
        DVE/ACT, single 1MB DMA to a blocked output tensor.

All input/output DRAM layouts are host-blocked so every DMA descriptor
moves >=2KB/partition contiguous runs (the single logical DMA queue is
descriptor-FIFO; emission order doubles as a prefetch schedule: the first
3 xt blocks are staged during phase P, xt(h1) blocks are staged through
B3(h0), x(h1) tiles through B1(h1)).

fp32r matmuls keep ~13 mantissa bits => logit noise ~0.08 => rel err ~6e-3.
"""
import sys

sys.path.insert(0, "/opt/trn_rl_repo")

import numpy as np

import concourse.bass as bass
import concourse.mybir as mybir
import concourse.tile as tile
from concourse import bacc
from concourse.bass import ts
from concourse.bass_utils import run_bass_kernel_spmd
from concourse.masks import make_identity

F32 = mybir.dt.float32
F32R = mybir.dt.float32r
F16 = mybir.dt.float16
AF = mybir.ActivationFunctionType
ALU = mybir.AluOpType

N = 8192
D = 512
NCORES = 8
MY_N = N // NCORES          # 1024 query rows per core
NJT = N // 128              # 64 j-tiles
NIH = MY_N // 512           # 2 i-halves
NBLK = N // 512             # 16 xt blocks

_NC_CACHE = None


def _build_nc():
    nc = bacc.Bacc(None, target_bir_lowering=False)

    xt = nc.dram_tensor("xt", [128, NBLK, 4, 512], F32R, kind="ExternalInput")  # X^T blocked
    xtmb = nc.dram_tensor("xtmb", [128, 2, 4, 512], F32R, kind="ExternalInput")  # X^T slice blocked
    x = nc.dram_tensor("x", [128, NBLK, 4, 512], F16, kind="ExternalInput")  # X blocked fp16
    wz = nc.dram_tensor("wz", [128, 4, 2, 512], F32R, kind="ExternalInput")  # Wq^T/Wk^T by d-chunk
    ob = nc.dram_tensor("ob", [128, 8, 512], F32, kind="ExternalOutput")     # blocked output

    with tile.TileContext(nc) as tc:
        with (
            tc.tile_pool(name="pool", bufs=1) as pool,          # persistent
            tc.tile_pool(name="stream", bufs=3) as stream,      # mt + xt blocks
            tc.tile_pool(name="big", bufs=1) as big,            # xtm then S-region
            tc.tile_pool(name="rpool", bufs=1) as rpool,        # wz then R
            tc.tile_pool(name="xs", bufs=3) as xsp,             # X tiles (B3)
            tc.tile_pool(name="workd", bufs=2) as workd,        # d
            tc.tile_pool(name="workp", bufs=2) as workp,        # p
            tc.tile_pool(name="osbp", bufs=1) as osbp,
            tc.tile_pool(name="ps_qk", bufs=3, space="PSUM") as ps_qk,
            tc.tile_pool(name="ps_o", bufs=1, space="PSUM") as ps_o,
            tc.tile_pool(name="ps_sum", bufs=1, space="PSUM") as ps_sum,
        ):
            # ---- constants ----
            ident = pool.tile([128, 128], F32)
            make_identity(nc, ident[:])
            ones_f32 = pool.tile([128, 2], F32)
            nc.vector.memset(ones_f32[:], 1.0)
            ones_col = pool.tile([128, 1], F16)    # rhs for row sums
            nc.vector.tensor_copy(ones_col[:], ones_f32[:, 0:1])
            ones_row_f32 = pool.tile([1, 128], F32)
            nc.vector.memset(ones_row_f32[:], 1.0)
            ones_row = pool.tile([1, 128], F32R)   # lhsT for broadcast
            nc.vector.tensor_copy(ones_row[:], ones_row_f32[:])

            # ---- PE warm-up: keep the PE busy during DMA staging so the
            #      HAM clock gate opens (K=8/8) before the first real matmul
            #      (same operand pattern as the proven finalize broadcast MM)
            for wu in range(16):
                wu_ps = ps_qk.tile([128, 128], F32, tag="qk")
                nc.tensor.matmul(
                    wu_ps[:], ones_row[:], ones_row[:], start=True, stop=True
                )

            # ---- staging: weights in 4 chunks, blocked xtm, xt prefetch ----
            wz_sb = rpool.tile([128, 4, 2, 512], F32R, tag="r")
            for dch in range(4):
                nc.sync.dma_start(wz_sb[:, dch], wz[:, dch])
            mt_sb = stream.tile([128, 4, 512], F32R, tag="stream")
            xtm_sb = big.tile([128, 2, 4, 512], F32R, tag="big")
            xt_pf = []  # (blk, tile) staged xt blocks
            nc.sync.dma_start(xtm_sb[:, 0], xtmb[:, 0])
            nc.sync.dma_start(xtm_sb[:, 1], xtmb[:, 1])
            for pfb in range(2):
                t_ = stream.tile([128, 4, 512], F32R, tag="stream")
                nc.sync.dma_start(t_[:], xt[:, pfb, :, :])
                xt_pf.append((pfb, t_))

            # ---- Phase P1: M^T = (Wq Wk^T)/sqrt(D), d-chunk outer so the
            #      matmuls pipeline against the weight-chunk DMAs ----
            scale = 1.0 / float(np.sqrt(D))
            mt_ps = []
            for fc in range(4):
                mtb = ps_o.tile([128, 512], F32, tag=f"o{fc}", name=f"mtb{fc}")
                mt_ps.append(mtb)
            for dch in range(4):
                for fc in range(4):
                    nc.tensor.matmul(
                        mt_ps[fc][:],
                        wz_sb[:, dch, 0, ts(fc, 128)],
                        wz_sb[:, dch, 1, :],
                        start=(dch == 0),
                        stop=(dch == 3),
                    )
            for fc in range(4):
                nc.scalar.activation(
                    mt_sb[:, fc, :], mt_ps[fc][:], AF.Copy, bias=0.0, scale=scale
                )

            # ---- Phase P2: R = M X_mine^T  (r_sb reuses wz_sb's memory) ----
            r_sb = rpool.tile([128, 4, MY_N], F32R, tag="r")
            for ih in range(NIH):
                for ech in range(4):
                    r_ps = ps_qk.tile([128, 512], F32, tag="qk")
                    for fch in range(4):
                        nc.tensor.matmul(
                            r_ps[:],
                            mt_sb[:, fch, ts(ech, 128)],
                            xtm_sb[:, ih, fch, :],
                            start=(fch == 0),
                            stop=(fch == 3),
                        )
                    nc.scalar.copy(r_sb[:, ech, ts(ih, 512)], r_ps[:])

            # ---- helpers ----
            def finalize_max(mx):
                """mx[128,512] -> b_sb[128,512] broadcast of per-i max."""
                mcol = pool.tile([128, 4], F32, tag="mcol")
                for c in range(4):
                    tp_ps = ps_qk.tile([128, 128], F32, tag="qk")
                    nc.tensor.transpose(tp_ps[:], mx[:, ts(c, 128)], ident[:])
                    nc.vector.reduce_max(
                        mcol[:, c : c + 1], tp_ps[:], axis=mybir.AxisListType.X
                    )
                mrow_ps = ps_qk.tile([1, 512], F32, tag="qk")
                for c in range(4):
                    nc.tensor.transpose(
                        mrow_ps[:, ts(c, 128)], mcol[:, c : c + 1], ident[:]
                    )
                mrow = pool.tile([1, 512], F32R, tag="mrow")
                nc.scalar.copy(mrow[:], mrow_ps[:])
                b_ps = ps_qk.tile([128, 512], F32, tag="qk")
                nc.tensor.matmul(b_ps[:], ones_row[:], mrow[:], start=True, stop=True)
                b_sb = pool.tile([128, 512], F32, tag="bsb")
                nc.scalar.copy(b_sb[:], b_ps[:])
                return b_sb

            def b1_phase(h, st, mx, pf):
                """Full B1 sweep for half h. pf: list of (blk, staged tile)."""
                for blk in range(NBLK):
                    if pf and pf[0][0] == blk:
                        xt_blk = pf.pop(0)[1]
                    else:
                        xt_blk = stream.tile([128, 4, 512], F32R, tag="stream")
                        nc.sync.dma_start(xt_blk[:], xt[:, blk, :, :])
                    if blk == 13:
                        # stage the first x block for the upcoming B3 phase
                        t_ = xsp.tile([128, 4, 512], F16, tag="x")
                        nc.sync.dma_start(t_[:], x[:, 0, :, :])
                        x_pf.append((0, t_))
                    for t in range(4):
                        jt = blk * 4 + t
                        s_ps = ps_qk.tile([128, 512], F32, tag="qk")
                        for e in range(4):
                            nc.tensor.matmul(
                                s_ps[:],
                                xt_blk[:, e, ts(t, 128)],
                                r_sb[:, e, ts(h, 512)],
                                start=(e == 0),
                                stop=(e == 3),
                            )
                        nc.scalar.copy(st[:, jt, :], s_ps[:])
                        if jt == 0:
                            nc.vector.tensor_copy(mx[:], s_ps[:])
                        else:
                            nc.vector.tensor_tensor(mx[:], mx[:], s_ps[:], op=ALU.max)

            def arm_b3():
                o_ps = []
                for c in range(4):
                    o_bank = ps_o.tile([128, 512], F32, tag=f"o{c}", name=f"o_bank{c}")
                    o_ps.append(o_bank)
                sum_ps = ps_sum.tile([128, 4], F32, tag="sum")
                return o_ps, sum_ps

            def b3_phase(h, st, b_sb, o_ps, sum_ps, pf):
                x_blk = None
                for jt in range(NJT):
                    if pf and pf[0][0] == jt:
                        x_blk = pf.pop(0)[1]
                    elif jt % 4 == 0:
                        x_blk = xsp.tile([128, 4, 512], F16, tag="x")
                        nc.sync.dma_start(x_blk[:], x[:, jt // 4, :, :])
                    if h == 0 and jt in (56, 60):
                        # stage B1(h1)'s first two xt blocks late in the x
                        # stream (1MB inserts; the 3-deep x ring absorbs them)
                        blk = (jt - 56) // 4
                        t_ = stream.tile([128, 4, 512], F32R, tag="stream")
                        nc.sync.dma_start(t_[:], xt[:, blk, :, :])
                        xt_pf.append((blk, t_))
                    d_t = workd.tile([128, 512], F32, tag="d")
                    nc.vector.tensor_tensor(
                        d_t[:], st[:, jt, :], b_sb[:], op=ALU.subtract
                    )
                    p_t = workp.tile([128, 512], F16, tag="p")
                    nc.scalar.activation(p_t[:], d_t[:], AF.Exp)
                    for c in range(4):
                        nc.tensor.matmul(
                            o_ps[c][:],
                            p_t[:, ts(c, 128)],
                            x_blk[:, jt % 4, :],
                            start=(jt == 0),
                            stop=(jt == NJT - 1),
                        )
                        nc.tensor.matmul(
                            sum_ps[:, c : c + 1],
                            p_t[:, ts(c, 128)],
                            ones_col[:],
                            start=(jt == 0 and c == 0),
                            stop=(jt == NJT - 1 and c == 3),
                        )

            def b4_drain(h, o_ps, sum_ps):
                """Normalize + evacuate o: copies split DVE/ACT so neither
                engine's FIFO stalls the next phase's subtract/exp chain."""
                rec = pool.tile([128, 4], F32, tag="rec")
                nc.vector.reciprocal(rec[:], sum_ps[:])
                o_sb = osbp.tile([128, 4, 512], F32, tag="osb")
                for c in range(4):
                    if c < 2:
                        nc.vector.tensor_scalar_mul(
                            o_sb[:, c, :], o_ps[c][:], rec[:, c : c + 1]
                        )
                    else:
                        nc.scalar.activation(
                            o_sb[:, c, :], o_ps[c][:], AF.Copy,
                            bias=0.0, scale=rec[:, c : c + 1],
                        )
                    if c == 1:
                        nc.sync.dma_start(
                            ob[:, h * 4 : h * 4 + 2, :], o_sb[:, 0:2, :]
                        )
                nc.sync.dma_start(ob[:, h * 4 + 2 : h * 4 + 4, :], o_sb[:, 2:4, :])

            # ---- main schedule ----
            x_pf = []
            mx = pool.tile([128, 512], F32, tag="mx")

            st0 = big.tile([128, NJT, 512], F32, tag="big")
            b1_phase(0, st0, mx, xt_pf)
            b_sb0 = finalize_max(mx)
            o_ps0, sum_ps0 = arm_b3()
            b3_phase(0, st0, b_sb0, o_ps0, sum_ps0, x_pf)
            b4_drain(0, o_ps0, sum_ps0)

            st1 = big.tile([128, NJT, 512], F32, tag="big")
            b1_phase(1, st1, mx, xt_pf)
            b_sb1 = finalize_max(mx)
            o_ps1, sum_ps1 = arm_b3()
            b3_phase(1, st1, b_sb1, o_ps1, sum_ps1, x_pf)
            b4_drain(1, o_ps1, sum_ps1)

    nc.compile()
    return nc


def _get_nc():
    global _NC_CACHE
    if _NC_CACHE is None:
        _NC_CACHE = _build_nc()
    return _NC_CACHE


def kernel(rotation_params, entangle_params, inputs, _trace=False, _trace_kwargs=None):
    X = np.ascontiguousarray(inputs, dtype=np.float32)
    Wq = np.ascontiguousarray(rotation_params, dtype=np.float32)
    Wk = np.ascontiguousarray(entangle_params, dtype=np.float32)
    XT = np.ascontiguousarray(X.T)
    # blocked layouts: [p, blk, c, j] with 8KiB (f32) / 4KiB (f16) runs/partition
    XTB = np.ascontiguousarray(
        XT.reshape(4, 128, 16, 512).transpose(1, 2, 0, 3)
    )
    X16B = np.ascontiguousarray(
        X.astype(np.float16).reshape(16, 4, 128, 512).transpose(2, 0, 1, 3)
    )
    # wz[p, dch, 0/1, f] = W{q,k}^T[dch*128+p, f]
    WZ = np.ascontiguousarray(
        np.stack(
            [
                Wq.T.reshape(4, 128, 512),
                Wk.T.reshape(4, 128, 512),
            ],
            axis=2,
        ).transpose(1, 0, 2, 3)
    )

    in_maps = []
    for c in range(NCORES):
        xtm = XT[:, c * MY_N : (c + 1) * MY_N]
        # xtmb[p, ih, fc, i] = xtm[fc*128+p, ih*512+i]
        XTMB = np.ascontiguousarray(
            xtm.reshape(4, 128, 2, 512).transpose(1, 2, 0, 3)
        )
        in_maps.append({"xt": XTB, "xtmb": XTMB, "x": X16B, "wz": WZ})

    nc = _get_nc()
    kw = {}
    if _trace:
        kw["trace"] = True
        kw.update(_trace_kwargs or {})
    br = run_bass_kernel_spmd(nc, in_maps, core_ids=list(range(NCORES)), **kw)
    # ob[p, hc, d] -> out[hc*128+p, d]
    out = np.concatenate(
        [r["ob"].transpose(1, 0, 2).reshape(MY_N, D) for r in br.results], axis=0
    )
    if _trace:
        return out, br
    return out


# revision 21
# speedup vs baseline: 1.0331x; 1.0331x over previous
"""Self-attention kernel for Trainium2 (8 NeuronCores, SPMD).

Problem: X[8192,512], Wq,Wk[512,512]:
    Q = X@Wq ; K = X@Wk ; S = softmax(Q K^T / sqrt(512)) ; out = S @ X

Sharding: rows of Q (query blocks of 1024) across 8 cores; K/V (=X) replicated.

Per-core dataflow (core owns query rows i in [c*1024, (c+1)*1024)):
  Phase P:  M^T = (Wq Wk^T)/sqrt(d)  [512,512]  (16 MMs, f32r, pipelined
            against the 4-chunk weights DMA via 4 concurrent PSUM groups)
            R   = M X_mine^T          [512,1024] (32 MMs; S^T = X R)
  Per i-half h (512 query columns):
    B1: for each j-tile (64): S^T tile [j=128, i=512] = sum_e X^T_tile.T R
        (4 accumulating f32r matmuls) -> ACT copy PSUM->SBUF,
        DVE running elementwise max -> mx[128,512]
    B2: partition-reduce mx via PE transpose + DVE reduce_max -> m[1,512];
        broadcast back to [128,512] via ones outer-product matmul
    B3: for each j-tile: d = S^T_t - B (DVE), exp (ACT, f16 out) -> P~;
        PE per i-chunk c: o[c,512] += P~[:,c].T @ X[j-tile] (N=512 fp16 MM)
        + a free N=1 MM on the same stationary:
        sum[c-chunk i,1] += P~[:,c].T @ ones  (row sums, already transposed)
    B4: DVE recip(sum[128,4]) -> per-bank scaled drain copies split across
        DVE/ACT, single 1MB DMA to a blocked output tensor.

All input/output DRAM layouts are host-blocked so every DMA descriptor
moves >=2KB/partition contiguous runs (the single logical DMA queue is
descriptor-FIFO; emission order doubles as a prefetch schedule: the first
3 xt blocks are staged during phase P, xt(h1) blocks are staged through
B3(h0), x(h1) tiles through B1(h1)).

fp32r matmuls keep ~13 mantissa bits => logit noise ~0.08 => rel err ~6e-3.
"""
import sys

sys.path.insert(0, "/opt/trn_rl_repo")

import numpy as np

import concourse.bass as bass
import concourse.mybir as mybir
import concourse.tile as tile
from concourse import bacc
from concourse.bass import ts
from concourse.bass_utils import run_bass_kernel_spmd
from concourse.masks import make_identity

F32 = mybir.dt.float32
F32R = mybir.dt.float32r
F16 = mybir.dt.float16
AF = mybir.ActivationFunctionType
ALU = mybir.AluOpType

N = 8192
D = 512
NCORES = 8
MY_N = N // NCORES          # 1024 query rows per core
NJT = N // 128              # 64 j-tiles
NIH = MY_N // 512           # 2 i-halves
NBLK = N // 512             # 16 xt blocks

_NC_CACHE = None


def _build_nc():
    nc = bacc.Bacc(None, target_bir_lowering=False)

    xt = nc.dram_tensor("xt", [128, NBLK, 4, 512], F32R, kind="ExternalInput")  # X^T blocked
    xtmb = nc.dram_tensor("xtmb", [128, 2, 4, 512], F32R, kind="ExternalInput")  # X^T slice blocked
    x = nc.dram_tensor("x", [128, NBLK, 4, 512], F16, kind="ExternalInput")  # X blocked fp16
    wz = nc.dram_tensor("wz", [128, 4, 2, 512], F32R, kind="ExternalInput")  # Wq^T/Wk^T by d-chunk
    ob = nc.dram_tensor("ob", [128, 8, 512], F32, kind="ExternalOutput")     # blocked output

    with tile.TileContext(nc) as tc:
        with (
            tc.tile_pool(name="pool", bufs=1) as pool,          # persistent
            tc.tile_pool(name="stream", bufs=3) as stream,      # mt + xt blocks
            tc.tile_pool(name="big", bufs=1) as big,            # xtm then S-region
            tc.tile_pool(name="rpool", bufs=1) as rpool,        # wz then R
            tc.tile_pool(name="xs", bufs=3) as xsp,             # X tiles (B3)
            tc.tile_pool(name="workd", bufs=3) as workd,        # d
            tc.tile_pool(name="workp", bufs=3) as workp,        # p
            tc.tile_pool(name="osbp", bufs=1) as osbp,
            tc.tile_pool(name="ps_qk", bufs=3, space="PSUM") as ps_qk,
            tc.tile_pool(name="ps_o", bufs=1, space="PSUM") as ps_o,
            tc.tile_pool(name="ps_sum", bufs=1, space="PSUM") as ps_sum,
        ):
            # ---- constants ----
            ident = pool.tile([128, 128], F32)
            make_identity(nc, ident[:])
            ones_f32 = pool.tile([128, 2], F32)
            nc.vector.memset(ones_f32[:], 1.0)
            ones_col = pool.tile([128, 1], F16)    # rhs for row sums
            nc.vector.tensor_copy(ones_col[:], ones_f32[:, 0:1])
            ones_row_f32 = pool.tile([1, 128], F32)
            nc.vector.memset(ones_row_f32[:], 1.0)
            ones_row = pool.tile([1, 128], F32R)   # lhsT for broadcast
            nc.vector.tensor_copy(ones_row[:], ones_row_f32[:])

            # ---- PE warm-up: keep the PE busy during DMA staging so the
            #      HAM clock gate opens (K=8/8) before the first real matmul
            #      (same operand pattern as the proven finalize broadcast MM)
            for wu in range(16):
                wu_ps = ps_qk.tile([128, 128], F32, tag="qk")
                nc.tensor.matmul(
                    wu_ps[:], ones_row[:], ones_row[:], start=True, stop=True
                )

            # ---- staging: weights in 4 chunks, blocked xtm, xt prefetch ----
            wz_sb = rpool.tile([128, 4, 2, 512], F32R, tag="r")
            for dch in range(4):
                nc.sync.dma_start(wz_sb[:, dch], wz[:, dch])
            mt_sb = stream.tile([128, 4, 512], F32R, tag="stream")
            xtm_sb = big.tile([128, 2, 4, 512], F32R, tag="big")
            xt_pf = []  # (blk, tile) staged xt blocks
            nc.sync.dma_start(xtm_sb[:, 0], xtmb[:, 0])
            nc.sync.dma_start(xtm_sb[:, 1], xtmb[:, 1])
            for pfb in range(2):
                t_ = stream.tile([128, 4, 512], F32R, tag="stream")
                nc.sync.dma_start(t_[:], xt[:, pfb, :, :])
                xt_pf.append((pfb, t_))

            # ---- Phase P1: M^T = (Wq Wk^T)/sqrt(D), d-chunk outer so the
            #      matmuls pipeline against the weight-chunk DMAs ----
            scale = 1.0 / float(np.sqrt(D))
            mt_ps = []
            for fc in range(4):
                mtb = ps_o.tile([128, 512], F32, tag=f"o{fc}", name=f"mtb{fc}")
                mt_ps.append(mtb)
            for dch in range(4):
                for fc in range(4):
                    nc.tensor.matmul(
                        mt_ps[fc][:],
                        wz_sb[:, dch, 0, ts(fc, 128)],
                        wz_sb[:, dch, 1, :],
                        start=(dch == 0),
                        stop=(dch == 3),
                    )
            for fc in range(4):
                nc.scalar.activation(
                    mt_sb[:, fc, :], mt_ps[fc][:], AF.Copy, bias=0.0, scale=scale
                )

            # ---- Phase P2: R = M X_mine^T  (r_sb reuses wz_sb's memory) ----
            r_sb = rpool.tile([128, 4, MY_N], F32R, tag="r")
            for ih in range(NIH):
                for ech in range(4):
                    r_ps = ps_qk.tile([128, 512], F32, tag="qk")
                    for fch in range(4):
                        nc.tensor.matmul(
                            r_ps[:],
                            mt_sb[:, fch, ts(ech, 128)],
                            xtm_sb[:, ih, fch, :],
                            start=(fch == 0),
                            stop=(fch == 3),
                        )
                    nc.scalar.copy(r_sb[:, ech, ts(ih, 512)], r_ps[:])

            # ---- helpers ----
            def finalize_max(mx):
                """mx[128,512] -> b_sb[128,512] broadcast of per-i max."""
                mcol = pool.tile([128, 4], F32, tag="mcol")
                for c in range(4):
                    tp_ps = ps_qk.tile([128, 128], F32, tag="qk")
                    nc.tensor.transpose(tp_ps[:], mx[:, ts(c, 128)], ident[:])
                    nc.vector.reduce_max(
                        mcol[:, c : c + 1], tp_ps[:], axis=mybir.AxisListType.X
                    )
                mrow_ps = ps_qk.tile([1, 512], F32, tag="qk")
                for c in range(4):
                    nc.tensor.transpose(
                        mrow_ps[:, ts(c, 128)], mcol[:, c : c + 1], ident[:]
                    )
                mrow = pool.tile([1, 512], F32R, tag="mrow")
                nc.scalar.copy(mrow[:], mrow_ps[:])
                b_ps = ps_qk.tile([128, 512], F32, tag="qk")
                nc.tensor.matmul(b_ps[:], ones_row[:], mrow[:], start=True, stop=True)
                b_sb = pool.tile([128, 512], F32, tag="bsb")
                nc.scalar.copy(b_sb[:], b_ps[:])
                return b_sb

            def b1_phase(h, st, mx, pf):
                """Full B1 sweep for half h. pf: list of (blk, staged tile)."""
                for blk in range(NBLK):
                    if pf and pf[0][0] == blk:
                        xt_blk = pf.pop(0)[1]
                    else:
                        xt_blk = stream.tile([128, 4, 512], F32R, tag="stream")
                        nc.sync.dma_start(xt_blk[:], xt[:, blk, :, :])
                    if blk == 13:
                        # stage the first x block for the upcoming B3 phase
                        t_ = xsp.tile([128, 4, 512], F16, tag="x")
                        nc.sync.dma_start(t_[:], x[:, 0, :, :])
                        x_pf.append((0, t_))
                    for t in range(4):
                        jt = blk * 4 + t
                        s_ps = ps_qk.tile([128, 512], F32, tag="qk")
                        for e in range(4):
                            nc.tensor.matmul(
                                s_ps[:],
                                xt_blk[:, e, ts(t, 128)],
                                r_sb[:, e, ts(h, 512)],
                                start=(e == 0),
                                stop=(e == 3),
                            )
                        nc.scalar.copy(st[:, jt, :], s_ps[:])
                        if jt == 0:
                            nc.vector.tensor_copy(mx[:], s_ps[:])
                        else:
                            nc.vector.tensor_tensor(mx[:], mx[:], s_ps[:], op=ALU.max)

            def arm_b3():
                o_ps = []
                for c in range(4):
                    o_bank = ps_o.tile([128, 512], F32, tag=f"o{c}", name=f"o_bank{c}")
                    o_ps.append(o_bank)
                sum_ps = ps_sum.tile([128, 4], F32, tag="sum")
                return o_ps, sum_ps

            def b3_phase(h, st, b_sb, o_ps, sum_ps, pf):
                x_blk = None
                for jt in range(NJT):
                    if pf and pf[0][0] == jt:
                        x_blk = pf.pop(0)[1]
                    elif jt % 4 == 0:
                        x_blk = xsp.tile([128, 4, 512], F16, tag="x")
                        nc.sync.dma_start(x_blk[:], x[:, jt // 4, :, :])
                    if h == 0 and jt == 56:
                        # stage B1(h1)'s first xt block late in the x stream
                        # (one 1MB insert; the 3-deep x ring absorbs the delay)
                        t_ = stream.tile([128, 4, 512], F32R, tag="stream")
                        nc.sync.dma_start(t_[:], xt[:, 0, :, :])
                        xt_pf.append((0, t_))
                    d_t = workd.tile([128, 512], F32, tag="d")
                    nc.vector.tensor_tensor(
                        d_t[:], st[:, jt, :], b_sb[:], op=ALU.subtract
                    )
                    p_t = workp.tile([128, 512], F16, tag="p")
                    nc.scalar.activation(p_t[:], d_t[:], AF.Exp)
                    for c in range(4):
                        nc.tensor.matmul(
                            o_ps[c][:],
                            p_t[:, ts(c, 128)],
                            x_blk[:, jt % 4, :],
                            start=(jt == 0),
                            stop=(jt == NJT - 1),
                        )
                        nc.tensor.matmul(
                            sum_ps[:, c : c + 1],
                            p_t[:, ts(c, 128)],
                            ones_col[:],
                            start=(jt == 0 and c == 0),
                            stop=(jt == NJT - 1 and c == 3),
                        )

            def b4_drain(h, o_ps, sum_ps):
                """Normalize + evacuate o: copies split DVE/ACT so neither
                engine's FIFO stalls the next phase's subtract/exp chain."""
                rec = pool.tile([128, 4], F32, tag="rec")
                nc.vector.reciprocal(rec[:], sum_ps[:])
                o_sb = osbp.tile([128, 4, 512], F32, tag="osb")
                for c in range(4):
                    if c < 2:
                        nc.vector.tensor_scalar_mul(
                            o_sb[:, c, :], o_ps[c][:], rec[:, c : c + 1]
                        )
                    else:
                        nc.scalar.activation(
                            o_sb[:, c, :], o_ps[c][:], AF.Copy,
                            bias=0.0, scale=rec[:, c : c + 1],
                        )
                    if c == 1:
                        nc.sync.dma_start(
                            ob[:, h * 4 : h * 4 + 2, :], o_sb[:, 0:2, :]
                        )
                nc.sync.dma_start(ob[:, h * 4 + 2 : h * 4 + 4, :], o_sb[:, 2:4, :])

            # ---- main schedule ----
            x_pf = []
            mx = pool.tile([128, 512], F32, tag="mx")

            st0 = big.tile([128, NJT, 512], F32, tag="big")
            b1_phase(0, st0, mx, xt_pf)
            b_sb0 = finalize_max(mx)
            o_ps0, sum_ps0 = arm_b3()
            b3_phase(0, st0, b_sb0, o_ps0, sum_ps0, x_pf)
            b4_drain(0, o_ps0, sum_ps0)

            st1 = big.tile([128, NJT, 512], F32, tag="big")
            b1_phase(1, st1, mx, xt_pf)
            b_sb1 = finalize_max(mx)
            o_ps1, sum_ps1 = arm_b3()
            b3_phase(1, st1, b_sb1, o_ps1, sum_ps1, x_pf)
            b4_drain(1, o_ps1, sum_ps1)

    nc.compile()
    return nc


def _get_nc():
    global _NC_CACHE
    if _NC_CACHE is None:
        _NC_CACHE = _build_nc()
    return _NC_CACHE


def kernel(rotation_params, entangle_params, inputs, _trace=False, _trace_kwargs=None):
    X = np.ascontiguousarray(inputs, dtype=np.float32)
    Wq = np.ascontiguousarray(rotation_params, dtype=np.float32)
    Wk = np.ascontiguousarray(entangle_params, dtype=np.float32)
    XT = np.ascontiguousarray(X.T)
    # blocked layouts: [p, blk, c, j] with 8KiB (f32) / 4KiB (f16) runs/partition
    XTB = np.ascontiguousarray(
        XT.reshape(4, 128, 16, 512).transpose(1, 2, 0, 3)
    )
    X16B = np.ascontiguousarray(
        X.astype(np.float16).reshape(16, 4, 128, 512).transpose(2, 0, 1, 3)
    )
    # wz[p, dch, 0/1, f] = W{q,k}^T[dch*128+p, f]
    WZ = np.ascontiguousarray(
        np.stack(
            [
                Wq.T.reshape(4, 128, 512),
                Wk.T.reshape(4, 128, 512),
            ],
            axis=2,
        ).transpose(1, 0, 2, 3)
    )

    in_maps = []
    for c in range(NCORES):
        xtm = XT[:, c * MY_N : (c + 1) * MY_N]
        # xtmb[p, ih, fc, i] = xtm[fc*128+p, ih*512+i]
        XTMB = np.ascontiguousarray(
            xtm.reshape(4, 128, 2, 512).transpose(1, 2, 0, 3)
        )
        in_maps.append({"xt": XTB, "xtmb": XTMB, "x": X16B, "wz": WZ})

    nc = _get_nc()
    kw = {}
    if _trace:
        kw["trace"] = True
        kw.update(_trace_kwargs or {})
    br = run_bass_kernel_spmd(nc, in_maps, core_ids=list(range(NCORES)), **kw)
    # ob[p, hc, d] -> out[hc*128+p, d]
    out = np.concatenate(
        [r["ob"].transpose(1, 0, 2).reshape(MY_N, D) for r in br.results], axis=0
    )
    if _trace:
        return out, br
    return out
